# revision 5
# baseline (speedup 1.0000x reference)
"""Trainium2 Bass kernel for bilinear texture sampling — v3 (proven gather shape).

Gather uses the hardware-verified indirect-DMA configuration: one int32 index
per partition ([128,1]) gathering 6 contiguous floats per partition
(texture row u, columns v0..v0+1).  Two calls per index column (rows u0 and
u0+1 via element_offset).  Everything else (index/weight math, bilinear
combine) is vectorized on the DVE.
"""

import os
import numpy as np

P = 128
N_CORES = 8
N = 4194304
NPC = N // N_CORES
W = 2048
K = 128            # samples per partition per chunk

_cached = {}


def _build(npc=NPC, k=K, n_cores=N_CORES):
    import concourse.bass as bass
    import concourse.tile as tile
    from concourse import bacc, mybir
    from contextlib import ExitStack

    f32 = mybir.dt.float32
    i32 = mybir.dt.int32
    Alu = mybir.AluOpType
    K = k
    n_chunks = npc // (P * K)
    assert n_chunks * P * K == npc

    nc = bacc.Bacc(
        "TRN2",
        target_bir_lowering=False,
        debug=False,
        enable_asserts=False,
        num_devices=n_cores,
    )
    tex_t = nc.dram_tensor("texture", [W * W, 3], f32, kind="ExternalInput")
    uv_t = nc.dram_tensor("uvs", [npc, 2], f32, kind="ExternalInput")
    col_t = nc.dram_tensor("colors", [npc, 3], f32, kind="ExternalOutput")

    tex_ap = tex_t.ap()
    uv_view = uv_t.ap().rearrange("(c p k) two -> c p (k two)", p=P, k=K)
    col_view = col_t.ap().rearrange("(c p k) three -> c p (k three)", p=P, k=K)

    with tile.TileContext(nc) as tc:
        with ExitStack() as ctx:
            uv_pool = ctx.enter_context(tc.tile_pool(name="uv", bufs=3))
            w_pool = ctx.enter_context(tc.tile_pool(name="work", bufs=2))
            q_pool = ctx.enter_context(tc.tile_pool(name="quads", bufs=3))
            o_pool = ctx.enter_context(tc.tile_pool(name="outs", bufs=3))

            for c in range(n_chunks):
                uv = uv_pool.tile([P, K, 2], f32)
                nc.sync.dma_start(uv[:].rearrange("p k two -> p (k two)"), uv_view[c])

                pv = w_pool.tile([P, K, 2], f32)
                nc.vector.tensor_scalar(pv[:], uv[:], 1.0, 1023.5, Alu.add, Alu.mult)
                it = w_pool.tile([P, K, 2], i32)
                nc.vector.tensor_copy(it[:], pv[:])
                fb = w_pool.tile([P, K, 2], f32)
                nc.vector.tensor_copy(fb[:], it[:])
                gt = w_pool.tile([P, K, 2], f32)
                nc.vector.tensor_tensor(out=gt[:], in0=fb[:], in1=pv[:], op=Alu.is_gt)
                f0 = w_pool.tile([P, K, 2], f32)
                nc.vector.tensor_tensor(out=f0[:], in0=fb[:], in1=gt[:], op=Alu.subtract)
                ab = w_pool.tile([P, K, 2], f32)
                nc.vector.tensor_tensor(out=ab[:], in0=pv[:], in1=f0[:], op=Alu.subtract)
                eq = w_pool.tile([P, K, 2], f32)
                nc.vector.tensor_scalar(eq[:], ab[:], 0.0, None, Alu.is_equal)
                abe = w_pool.tile([P, K, 2], f32)
                nc.vector.tensor_tensor(out=abe[:], in0=ab[:], in1=eq[:], op=Alu.add)

                idxf = w_pool.tile([P, K], f32)
                nc.vector.scalar_tensor_tensor(
                    out=idxf[:], in0=f0[:, :, 0], scalar=2048.0, in1=f0[:, :, 1],
                    op0=Alu.mult, op1=Alu.add,
                )
                idx = w_pool.tile([P, K], i32)
                nc.vector.tensor_copy(idx[:], idxf[:])

                qa = q_pool.tile([P, K, 6], f32)
                qb = q_pool.tile([P, K, 6], f32)
                for j in range(K):
                    nc.gpsimd.indirect_dma_start(
                        out=qa[:, j, :], out_offset=None, in_=tex_ap[:],
                        in_offset=bass.IndirectOffsetOnAxis(ap=idx[:, j : j + 1], axis=0),
                    )
                    nc.gpsimd.indirect_dma_start(
                        out=qb[:, j, :], out_offset=None, in_=tex_ap[:],
                        in_offset=bass.IndirectOffsetOnAxis(ap=idx[:, j : j + 1], axis=0),
                        element_offset=W * 3,
                    )

                a_b = abe[:, :, 0:1].to_broadcast([P, K, 6])
                d = w_pool.tile([P, K, 6], f32)
                nc.vector.tensor_tensor(out=d[:], in0=qa[:], in1=qb[:], op=Alu.subtract)
                dm = w_pool.tile([P, K, 6], f32)
                nc.vector.tensor_tensor(out=dm[:], in0=d[:], in1=a_b, op=Alu.mult)
                m = w_pool.tile([P, K, 6], f32)
                nc.vector.tensor_tensor(out=m[:], in0=dm[:], in1=qb[:], op=Alu.add)

                b_b = abe[:, :, 1:2].to_broadcast([P, K, 3])
                e = w_pool.tile([P, K, 3], f32)
                nc.vector.tensor_tensor(
                    out=e[:], in0=m[:, :, 0:3], in1=m[:, :, 3:6], op=Alu.subtract
                )
                eb = w_pool.tile([P, K, 3], f32)
                nc.vector.tensor_tensor(out=eb[:], in0=e[:], in1=b_b, op=Alu.mult)
                ot = o_pool.tile([P, K, 3], f32)
                nc.vector.tensor_tensor(out=ot[:], in0=eb[:], in1=m[:, :, 3:6], op=Alu.add)

                nc.sync.dma_start(col_view[c], ot[:].rearrange("p k three -> p (k three)"))

    nc.compile()
    return nc


def kernel(uvs: np.ndarray, texture: np.ndarray) -> np.ndarray:
    from concourse import bass_utils

    if "nc" not in _cached:
        _cached["nc"] = _build()
    nc = _cached["nc"]

    tex_flat = np.ascontiguousarray(texture.reshape(W * W, 3), dtype=np.float32)
    uvs = np.ascontiguousarray(uvs, dtype=np.float32)
    in_maps = [
        {"texture": tex_flat, "uvs": uvs[g * NPC : (g + 1) * NPC]}
        for g in range(N_CORES)
    ]
    res = bass_utils.run_bass_kernel_spmd(
        nc, in_maps, core_ids=list(range(N_CORES)),
        trace=bool(int(os.environ.get("DIFFTEX_TRACE", "0"))),
    )
    _cached["last_results"] = res
    out = np.concatenate([r["colors"] for r in res.results], axis=0)
    return out
